# revision 6
# baseline (speedup 1.0000x reference)
"""Trainium2 Bass kernel for the GRU classifier (nn_Classifiergru).

kernel(**inputs) takes the FULL inputs (as in reference.setup_inputs())
and returns the FULL [1, 1, 1] float32 output. Per the sharding hint
there is no useful parallelism at batch=1/hidden=100: the same fused
single-core kernel is replicated across all 8 NeuronCores and core 0's
output is returned.

The 550-step recurrence is latency-bound (tiny tensors, sequential
dependence), so the design minimizes the serial chain per step: two
ScalarE ops (sigmoid, tanh) + one small DVE blend + two matmuls, with
everything else scheduled off the critical path. Measured on HW (For_i
hardware-loop amortization): ~621 us vs ~1319 us for the previous
6-op/5-hop-per-step version.

Per step:
  PE : psA[.,2] = gi_rz(t)  (identity-stationary matmul, runs early)
       psA += W_rz^T e ; psB = W_n^T e (+b_hh_n)   (early, during prev ACT)
       psA += W_rz^T q ; psB += W_n^T q            (gates the next step)
  ACT: rz = sigmoid(psA)               [100,2] -> SBUF
       n  = tanh(psB*r + gi_n(t))      scale=r, bias=gi_n -> PSUM
  DVE: f  = 1 - z
       e' = (q + e) * z                (fused h materialization)
       q' = n * f                      -> fp16 state
State is split h = q + e (two fp16 columns); the e-side matmuls of the
next step run during the ACT window, only the q-side gates it.
"""

import sys
from contextlib import ExitStack

import numpy as np

for _p in ("/opt/trn_rl_repo", "/root/.axon_site/_ro/trn_rl_repo"):
    if _p not in sys.path:
        sys.path.append(_p)

import concourse.bass as bass
import concourse.bacc as bacc
import concourse.tile as tile
import concourse.mybir as mybir
from concourse.bass_utils import run_bass_kernel_spmd

F32 = mybir.dt.float32
F16 = mybir.dt.float16
AF = mybir.ActivationFunctionType
ALU = mybir.AluOpType

VOCAB = 100
EMBED = 10
MID = 100
SEQ = 550
N_CORES = 8


def _prep_inputs(x, hidden, embed_table, w_ih, w_hh, b_ih, b_hh, fc_w, fc_b):
    """Pure layout transforms of the reference inputs -> device input dict."""
    x = np.asarray(x).astype(np.int64)
    T = x.shape[0]
    V2 = VOCAB + 2  # 101 one-hot rows + 1 ones row (b_ih folding)

    oh = np.zeros((V2, T), dtype=np.float32)
    oh[x, np.arange(T)] = 1.0
    oh[VOCAB + 1, :] = 1.0

    tblT_ext = np.zeros((EMBED + 1, V2), dtype=np.float32)
    tblT_ext[:EMBED, : VOCAB + 1] = np.asarray(embed_table, np.float32).T
    tblT_ext[EMBED, VOCAB + 1] = 1.0

    w_ihT_b = np.concatenate(
        [np.asarray(w_ih, np.float32).T, np.asarray(b_ih, np.float32)[None, :]], axis=0
    )

    # [101, 3*128]: gate blocks r, z, n; rows 0..99 = w_hh_g.T (zero-padded
    # to 128 cols for fast weight load); row 100 = 0 except the n block,
    # which carries b_hh_n (rides on the e column's constant 1 in row 100).
    whh = np.asarray(w_hh, np.float32)
    whh3 = np.zeros((MID + 1, 3 * 128), dtype=np.float16)
    for g in range(3):
        whh3[:MID, g * 128 : g * 128 + MID] = whh[g * MID : (g + 1) * MID].T
    whh3[MID, 256 : 256 + MID] = np.asarray(b_hh, np.float16)[2 * MID :]

    bhh = np.asarray(b_hh, np.float32)
    bhh2 = np.stack([bhh[:MID], bhh[MID : 2 * MID]], axis=1)

    id16 = np.zeros((MID, 128), dtype=np.float16)
    id16[np.arange(MID), np.arange(MID)] = 1.0

    h0 = np.asarray(hidden, np.float32).reshape(MID, 1)
    qe_init = np.zeros((MID + 1, 2), dtype=np.float16)
    qe_init[:MID, 0] = h0[:, 0]
    qe_init[MID, 1] = 1.0

    fcw = np.asarray(fc_w, np.float32).reshape(1, MID).T.copy()
    fcb = np.asarray(fc_b, np.float32).reshape(1, 1)

    # Pack the small tensors into three DMA payloads (each DMA has a
    # large fixed issue cost, so fewer transfers beat smaller ones):
    #   packA fp32 [11, 402]  = tblT_ext | w_ihT_b
    #   packB fp32 [100, 4]   = bhh2 | fcw | fcb(row 0)
    #   packC fp16 [101, 514] = whh3 | id16(rows 0..99) | qe_init\n    #   (whh3 at offset 0 and id16 at byte 768 keep the stationary weight\n    #   blocks 64B-aligned for the fast-weight-load path)
    packA = np.concatenate([tblT_ext, w_ihT_b], axis=1)
    packB = np.zeros((MID, 4), dtype=np.float32)
    packB[:, 0:2] = bhh2
    packB[:, 2:3] = fcw
    packB[0, 3] = fcb[0, 0]
    packC = np.zeros((MID + 1, 514), dtype=np.float16)
    packC[:, 0:384] = whh3
    packC[:MID, 384:512] = id16
    packC[:, 512:514] = qe_init

    return {
        "oh": np.ascontiguousarray(oh),
        "packA": np.ascontiguousarray(packA),
        "packB": np.ascontiguousarray(packB),
        "packC": np.ascontiguousarray(packC),
    }


def _build_nc(T=SEQ, reps=1):
    V2 = VOCAB + 2
    nc = bacc.Bacc()

    oh_d = nc.declare_dram_parameter("oh", [V2, T], F32, isOutput=False)
    pA_d = nc.declare_dram_parameter("packA", [EMBED + 1, 402], F32, isOutput=False)
    pB_d = nc.declare_dram_parameter("packB", [MID, 4], F32, isOutput=False)
    pC_d = nc.declare_dram_parameter("packC", [MID + 1, 514], F16, isOutput=False)
    out_d = nc.declare_dram_parameter("out", [1, 1], F32, isOutput=True)

    with ExitStack() as ctx:
        tc = ctx.enter_context(tile.TileContext(nc))
        cpool = ctx.enter_context(tc.tile_pool(name="const", bufs=1))
        wpool = ctx.enter_context(tc.tile_pool(name="work", bufs=4))
        pA = ctx.enter_context(tc.tile_pool(name="psA", bufs=2, space="PSUM"))
        pB = ctx.enter_context(tc.tile_pool(name="psB", bufs=2, space="PSUM"))
        pN = ctx.enter_context(tc.tile_pool(name="psN", bufs=2, space="PSUM"))
        prepool = ctx.enter_context(tc.tile_pool(name="pre", bufs=2, space="PSUM"))

        # ---- load constants/weights (4 DMAs, dependency-ordered) ----
        packA = cpool.tile([EMBED + 1, 402], F32, tag="packA")
        nc.sync.dma_start(packA[:], pA_d[:])
        packB = cpool.tile([MID, 4], F32, tag="packB")
        nc.sync.dma_start(packB[:], pB_d[:])
        oh = cpool.tile([V2, T], F32, tag="oh")
        nc.sync.dma_start(oh[:], oh_d[:])
        packC = cpool.tile([MID + 1, 514], F16, tag="packC")
        nc.sync.dma_start(packC[:], pC_d[:])

        tblT = packA[:, 0:V2]
        wih = packA[:, V2 : V2 + 3 * MID]
        bhh2 = packB[:, 0:2]
        fcw = packB[:, 2:3]
        fcb = packB[0:1, 3:4]
        whh16 = packC[:, 0:384]
        id16 = packC[0:MID, 384:512]

        qe_a = cpool.tile([MID + 1, 2], F16, tag="qe_a")
        qe_b = cpool.tile([MID + 1, 2], F16, tag="qe_b")
        qe = [qe_a, qe_b]
        nc.vector.tensor_copy(qe_a[:], packC[:, 512:514])
        nc.vector.tensor_copy(qe_b[:], packC[:, 512:514])

        # ---- prelude: GI3 [100, T, 2] fp16 (r, z incl. b_hh), GIn fp32 ----
        GI3 = cpool.tile([MID, T, 2], F16, tag="gi3")
        GIn = cpool.tile([MID, T], F32, tag="gin")
        for g in range(3):
            mg_ps = prepool.tile([V2, MID], F32, tag="pre")
            nc.tensor.matmul(mg_ps[:], tblT, wih[:, g * MID : (g + 1) * MID])
            mg = cpool.tile([V2, MID], F32, tag=f"mg{g}")
            nc.vector.tensor_copy(mg[:], mg_ps[:])

            for c0 in range(0, T, 512):
                c1 = min(c0 + 512, T)
                gi_ps = prepool.tile([MID, c1 - c0], F32, tag="pre")
                nc.tensor.matmul(gi_ps[:], mg[:], oh[:, c0:c1])
                if g < 2:
                    nc.vector.tensor_scalar_add(
                        GI3[:, c0:c1, g], gi_ps[:], bhh2[:, g : g + 1]
                    )
                else:
                    nc.vector.tensor_copy(GIn[:, c0:c1], gi_ps[:])

        # ---- recurrence ----
        def step(t):
            sin = qe[t % 2]
            sout = qe[(t + 1) % 2]

            psA = pA.tile([128, 2], F32, tag="psA")
            psB = pB.tile([128, 1], F32, tag="psB")

            # gi injection + e-side matmuls run during the previous step's
            # ACT/DVE window; the q-side matmuls gate the next step.
            nc.tensor.matmul(psA[:, 0:2], id16, GI3[:, t, :], start=True, stop=False)
            nc.tensor.matmul(psA[:, 0:1], whh16[:, 0:128], sin[:, 1:2], start=False, stop=False)
            nc.tensor.matmul(psA[:, 1:2], whh16[:, 128:256], sin[:, 1:2], start=False, stop=False)
            nc.tensor.matmul(psB[:, 0:1], whh16[:, 256:384], sin[:, 1:2], start=True, stop=False)
            nc.tensor.matmul(psA[:, 0:1], whh16[:, 0:128], sin[:, 0:1], start=False, stop=True)
            nc.tensor.matmul(psA[:, 1:2], whh16[:, 128:256], sin[:, 0:1], start=False, stop=True)
            nc.tensor.matmul(psB[:, 0:1], whh16[:, 256:384], sin[:, 0:1], start=False, stop=True)

            rzf = wpool.tile([MID, 2], F32, tag="rzf")
            nc.scalar.activation(rzf[:], psA[0:MID, :], AF.Sigmoid)

            # DVE (off the ACT chain): f = 1-z ; e' = (q+e)*z
            f_t = wpool.tile([MID, 1], F32, tag="f")
            nc.vector.tensor_scalar(f_t[:], rzf[:, 1:2], -1.0, 1.0, ALU.mult, ALU.add)
            nc.vector.scalar_tensor_tensor(
                sout[0:MID, 1:2], sin[0:MID, 0:1], sin[0:MID, 1:2], rzf[:, 1:2],
                ALU.add, ALU.mult,
            )

            n_ps = pN.tile([MID, 1], F32, tag="n")
            nc.scalar.activation(
                n_ps[:], psB[0:MID, :], AF.Tanh, bias=GIn[:, t : t + 1], scale=rzf[:, 0:1]
            )
            # q' = n * f on the DVE, straight to fp16 state
            nc.vector.tensor_scalar(
                sout[0:MID, 0:1], n_ps[:], f_t[:], None, ALU.mult
            )

        if reps == 1:
            for t in range(T):
                step(t)
        else:
            with tc.For_i(0, reps):
                for t in range(T):
                    step(t)

        # ---- epilogue: out = sigmoid(relu(q+e) @ fc_w.T + fc_b) ----
        sfin = qe[T % 2]
        hfin = wpool.tile([MID, 1], F32, tag="hfin")
        nc.vector.tensor_tensor(hfin[:], sfin[0:MID, 0:1], sfin[0:MID, 1:2], ALU.add)
        rh = wpool.tile([MID, 1], F32, tag="rh")
        nc.vector.tensor_scalar_max(rh[:], hfin[:], 0.0)
        po = prepool.tile([1, 1], F32, tag="pre")
        nc.tensor.matmul(po[:], rh[:], fcw)
        ot = wpool.tile([1, 1], F32, tag="ot")
        nc.scalar.activation(ot[:], po[:], AF.Sigmoid, bias=fcb)
        nc.sync.dma_start(out_d[:], ot[:])

    nc.finalize()
    return nc


_NC_CACHE = {}


def _get_nc(T=SEQ, reps=1):
    key = (T, reps)
    if key not in _NC_CACHE:
        _NC_CACHE[key] = _build_nc(T, reps)
    return _NC_CACHE[key]


def kernel(x, hidden, embed_table, w_ih, w_hh, b_ih, b_hh, fc_w, fc_b, **_kwargs):
    dev_in = _prep_inputs(x, hidden, embed_table, w_ih, w_hh, b_ih, b_hh, fc_w, fc_b)
    nc = _get_nc(SEQ)
    in_maps = [dev_in for _ in range(N_CORES)]
    res = run_bass_kernel_spmd(nc, in_maps, list(range(N_CORES)))
    out = np.asarray(res.results[0]["out"], dtype=np.float32).reshape(1, 1, 1)
    return out
